# revision 7
# baseline (speedup 1.0000x reference)
"""Trainium2 Bass kernel for nn_Fast2Order_DE_Conv.

Math: out[b,o,ho,wo] = sum_{c,i,j} W[o, c*81+i*9+j] * p_i * p_j with
p_i = x[b, c, ho+di, wo+dj] (i = di*3+dj, 3x3 unfold of a 16-channel 64x64
image; output 62x62).

Algorithm: change the quadratic-feature basis from products p_i*p_j to
squares {p_i^2, (p_i+p_j)^2, i<j} (45 per channel, 720 total) and fold the
basis change into W on the host (W2 = W * M^-1).  On-chip, per spatial tile
of 512 locations:

    selection matmul (PE, f32r):  s = AselT.T @ x_unfold   [720 rows]
    square          (ACT/DVE):    g = s^2, PSUM -> SBUF f32r
    main matmul     (PE, f32r):   out += W2T.T @ g, accumulated in PSUM

All matmuls use float32r (fp32 RNE-rounded to 11 mantissa bits, full PE
rate at moving dim >= 256).  Inputs are pre-rounded to the f32r grid on the
host so DMA loads feed the PE directly.  The 3x3 unfold itself is free: it
is expressed in the DMA access pattern (overlapping windows of the padded
l' = ho*64+wo layout).

Sharding: data-parallel over batch, 2 batches per core on 8 cores; W-side
constants are replicated.  Output gathered by simple concatenation.
"""

import functools

import numpy as np

import concourse.bacc as bacc
import concourse.mybir as mybir
from concourse.tile import TileContext
from concourse.bass_utils import run_bass_kernel_spmd

B, C, H, WIDTH = 16, 16, 64, 64
O = 128
HO = WO = 62
N_CORES = 8
B_LOC = B // N_CORES
PAIRS = [(i, j) for i in range(9) for j in range(i, 9)]  # 45
ROW_TILES = [(0, 8), (8, 8), (16, 8), (24, 8), (32, 8), (40, 8), (48, 8), (56, 6)]
NCHUNK = 6  # g chunks of 120 rows (720 total)
GC = 120


def _round_f32r(a: np.ndarray) -> np.ndarray:
    """Round fp32 values to the f32r grid (RNE at 12 low mantissa bits)."""
    a = np.ascontiguousarray(a, dtype=np.float32)
    bits = a.view(np.uint32).astype(np.uint64)
    half, mask = np.uint64(0x800), np.uint64(0xFFF)
    lsb = (bits >> np.uint64(12)) & np.uint64(1)
    out = ((bits + half - np.uint64(1) + lsb) & ~mask).astype(np.uint32)
    return out.view(np.float32).reshape(a.shape)


def _build_consts(Wf: np.ndarray):
    """W (128, 1296) -> (AselT [72, 360] f32, W2T [720, 128] f32, f32r grid)."""
    Wt = np.asarray(Wf, dtype=np.float64).reshape(O, C, 9, 9)
    Wsym = Wt + Wt.transpose(0, 1, 3, 2)
    W2 = np.zeros((O, 720))
    for c in range(C):
        for pi, (i, j) in enumerate(PAIRS):
            f = c * 45 + pi
            if i == j:
                W2[:, f] = Wt[:, c, i, i] - 0.5 * (
                    Wsym[:, c, i, :].sum(-1) - 2.0 * Wt[:, c, i, i]
                )
            else:
                W2[:, f] = 0.5 * Wsym[:, c, i, j]
    # x-row layout on chip: row = i*8 + c_local (i = di*3+dj kernel position)
    AselT = np.zeros((72, 360), dtype=np.float32)
    for cl in range(8):
        for pi, (i, j) in enumerate(PAIRS):
            g = cl * 45 + pi
            AselT[i * 8 + cl, g] += 1.0
            if i != j:
                AselT[j * 8 + cl, g] += 1.0
    W2T = np.ascontiguousarray(W2.T)  # [720, 128]
    return AselT, _round_f32r(W2T)


def _x_window_ap(x_d, b: int, h: int, ho0: int, di: int, lt_load: int):
    """Source AP for one di of the unfold load: (dj, c, l) nesting matching
    target partitions (di*3+dj)*8 + c, free dim = padded l' = ho*64+wo."""
    ap = x_d[b, h * 8 : (h + 1) * 8, ho0 + di, 0:3].unsqueeze(-1)
    v = ap.ap
    v[0] = [1, 3]
    v[1] = [H * WIDTH, 8]
    v[2] = [1, lt_load]
    return ap


def build_nc():
    f32, f32r = mybir.dt.float32, mybir.dt.float32r
    nc = bacc.Bacc("TRN2", target_bir_lowering=False)
    x_d = nc.dram_tensor("x_loc", [B_LOC, C, H, WIDTH], f32r, kind="ExternalInput")
    a_d = nc.dram_tensor("aselT", [72, 360], f32r, kind="ExternalInput")
    w_d = nc.dram_tensor("w2T", [720, O], f32r, kind="ExternalInput")
    o_d = nc.dram_tensor("out_loc", [B_LOC, O, HO, WO], f32, kind="ExternalOutput")

    with TileContext(nc) as tc:
        with (
            tc.tile_pool(name="const", bufs=1) as cpool,
            tc.tile_pool(name="xin", bufs=3) as xpool,
            tc.tile_pool(name="gbuf", bufs=4) as gpool,
            tc.tile_pool(name="obuf", bufs=3) as opool,
            tc.tile_pool(name="ps_sel", bufs=3, space="PSUM") as pspool,
            tc.tile_pool(name="ps_out", bufs=2, space="PSUM") as popool,
        ):
            a_r = cpool.tile([72, 360], f32r, tag="a_r")
            nc.sync.dma_start(a_r[:], a_d[:])
            w_r = cpool.tile([GC, NCHUNK, O], f32r, tag="w_r")
            nc.sync.dma_start(w_r[:], w_d[:].rearrange("(k p) o -> p k o", p=GC))

            for b in range(B_LOC):
                for ho0, nr in ROW_TILES:
                    lt = nr * 64
                    xr = []
                    for h in range(2):
                        x_t = xpool.tile([72, 512], f32r, tag=f"x{h}")
                        for di in range(3):
                            # last needed l' column is (nr-1)*64 + 61; clamp so
                            # the deepest window (dj=2) stays inside the channel
                            lt_load = min(lt, H * WIDTH - (ho0 + di) * 64 - 2)
                            nc.sync.dma_start(
                                x_t[di * 24 : (di + 1) * 24, :lt_load],
                                _x_window_ap(x_d, b, h, ho0, di, lt_load),
                            )
                            if lt_load < lt:
                                # pad columns feed discarded outputs; fill with
                                # arbitrary valid f32r data to keep reads clean
                                nc.sync.dma_start(
                                    x_t[di * 24 : (di + 1) * 24, lt_load:lt],
                                    _x_window_ap(x_d, b, h, 0, 0, lt - lt_load),
                                )
                        xr.append(x_t)
                    ps_o = popool.tile([O, 512], f32, tag="ps_o")
                    for kk in range(NCHUNK):
                        h, k = divmod(kk, 3)
                        ps_s = pspool.tile([GC, 512], f32, tag="ps_s")
                        nc.tensor.matmul(
                            ps_s[:, :lt],
                            a_r[:, k * GC : (k + 1) * GC],
                            xr[h][:, :lt],
                            start=True,
                            stop=True,
                        )
                        g_t = gpool.tile([GC, 512], f32r, tag="g")
                        nc.scalar.square(g_t[:, :lt], ps_s[:, :lt])
                        nc.tensor.matmul(
                            ps_o[:, :lt],
                            w_r[:, kk, :],
                            g_t[:, :lt],
                            start=(kk == 0),
                            stop=(kk == NCHUNK - 1),
                        )
                    o_t = opool.tile([O, 512], f32, tag="o")
                    nc.vector.tensor_copy(o_t[:, :lt], ps_o[:, :lt])
                    nc.sync.dma_start(
                        o_d[b, :, ho0 : ho0 + nr, :],
                        o_t[:].rearrange("o (r w) -> o r w", w=64)[:, :nr, :WO],
                    )
    nc.compile()
    return nc


@functools.lru_cache(maxsize=1)
def _cached_nc():
    return build_nc()


def kernel(x: np.ndarray, W: np.ndarray, _trace: bool = False):
    x = np.asarray(x, dtype=np.float32)
    W = np.asarray(W, dtype=np.float32)
    AselT, W2T = _build_consts(W)
    x_r = _round_f32r(x)

    nc = _cached_nc()
    in_maps = [
        {
            "x_loc": np.ascontiguousarray(x_r[k * B_LOC : (k + 1) * B_LOC]),
            "aselT": AselT,
            "w2T": W2T,
        }
        for k in range(N_CORES)
    ]
    r = run_bass_kernel_spmd(nc, in_maps, core_ids=list(range(N_CORES)), trace=_trace)
    out = np.concatenate([m["out_loc"] for m in r.results], axis=0)
    if _trace:
        kernel.last_result = r
    return out


if __name__ == "__main__":
    rng = np.random.default_rng(0)
    x = rng.standard_normal((B, C, H, WIDTH), dtype=np.float32)
    W = rng.standard_normal((O, C * 81), dtype=np.float32)
    out = kernel(x, W)
    print("out shape", out.shape, out.dtype)


# revision 33
# speedup vs baseline: 1.0149x; 1.0149x over previous
"""Trainium2 Bass kernel for nn_Fast2Order_DE_Conv.

Math: out[b,o,ho,wo] = sum_{c,i,j} W[o, c*81+i*9+j] * p_i * p_j with
p_i = x[b, c, ho+di, wo+dj] (i = di*3+dj, 3x3 unfold of a 16-channel 64x64
image; output 62x62).

Algorithm: change the quadratic-feature basis from products p_i*p_j to
squares {p_i^2, (p_i+p_j)^2, i<j} (45 per channel, 720 total) and fold the
basis change into W on the host (W2 = W * M^-1).  On-chip, per spatial tile
of 512 locations:

    selection matmul (PE, f32r):  s = AselT.T @ x_unfold   [720 rows]
    square          (ACT/DVE):    g = s^2, PSUM -> SBUF f32r
    main matmul     (PE, f32r):   out += W2T.T @ g, accumulated in PSUM

All matmuls use float32r (fp32 RNE-rounded to 11 mantissa bits, full PE
rate at moving dim >= 256).  Inputs are pre-rounded to the f32r grid on the
host so DMA loads feed the PE directly.  The 3x3 unfold itself is free: it
is expressed in the DMA access pattern (overlapping windows of the padded
l' = ho*64+wo layout).

Sharding: data-parallel over batch, 2 batches per core on 8 cores; W-side
constants are replicated.  Output gathered by simple concatenation.
"""

import functools

import numpy as np

import concourse.bacc as bacc
import concourse.mybir as mybir
from concourse.tile import TileContext
from concourse.bass_utils import run_bass_kernel_spmd

B, C, H, WIDTH = 16, 16, 64, 64
O = 128
HO = WO = 62
N_CORES = 8
B_LOC = B // N_CORES
PAIRS = [(i, j) for i in range(9) for j in range(i, 9)]  # 45
ROW_TILES = [(0, 8), (8, 8), (16, 8), (24, 8), (32, 8), (40, 8), (48, 8), (56, 6)]
NCHUNK = 6  # g chunks of 120 rows (720 total)
GC = 120


def _round_f32r(a: np.ndarray) -> np.ndarray:
    """Round fp32 values to the f32r grid (RNE at 12 low mantissa bits)."""
    a = np.ascontiguousarray(a, dtype=np.float32)
    bits = a.view(np.uint32).astype(np.uint64)
    half, mask = np.uint64(0x800), np.uint64(0xFFF)
    lsb = (bits >> np.uint64(12)) & np.uint64(1)
    out = ((bits + half - np.uint64(1) + lsb) & ~mask).astype(np.uint32)
    return out.view(np.float32).reshape(a.shape)


def _build_consts(Wf: np.ndarray):
    """W (128, 1296) -> (AselT [72, 360] f32, W2T [720, 128] f32, f32r grid)."""
    Wt = np.asarray(Wf, dtype=np.float64).reshape(O, C, 9, 9)
    Wsym = Wt + Wt.transpose(0, 1, 3, 2)
    W2 = np.zeros((O, 720))
    for c in range(C):
        for pi, (i, j) in enumerate(PAIRS):
            f = c * 45 + pi
            if i == j:
                W2[:, f] = Wt[:, c, i, i] - 0.5 * (
                    Wsym[:, c, i, :].sum(-1) - 2.0 * Wt[:, c, i, i]
                )
            else:
                W2[:, f] = 0.5 * Wsym[:, c, i, j]
    # x-row layout on chip: row = i*8 + c_local (i = di*3+dj kernel position)
    AselT = np.zeros((72, 360), dtype=np.float32)
    for cl in range(8):
        for pi, (i, j) in enumerate(PAIRS):
            g = cl * 45 + pi
            AselT[i * 8 + cl, g] += 1.0
            if i != j:
                AselT[j * 8 + cl, g] += 1.0
    W2T = np.ascontiguousarray(W2.T)  # [720, 128]
    return AselT, _round_f32r(W2T)


def _x_window_ap(x_d, b: int, h: int, ho0: int, di: int, lt_load: int):
    """Source AP for one di of the unfold load: (dj, c, l) nesting matching
    target partitions (di*3+dj)*8 + c, free dim = padded l' = ho*64+wo."""
    ap = x_d[b, h * 8 : (h + 1) * 8, ho0 + di, 0:3].unsqueeze(-1)
    v = ap.ap
    v[0] = [1, 3]
    v[1] = [H * WIDTH, 8]
    v[2] = [1, lt_load]
    return ap


def build_nc():
    f32, f32r = mybir.dt.float32, mybir.dt.float32r
    nc = bacc.Bacc("TRN2", target_bir_lowering=False)
    x_d = nc.dram_tensor("x_loc", [B_LOC, C, H, WIDTH], f32r, kind="ExternalInput")
    a_d = nc.dram_tensor("aselT", [72, 360], f32r, kind="ExternalInput")
    w_d = nc.dram_tensor("w2T", [720, O], f32r, kind="ExternalInput")
    o_d = nc.dram_tensor("out_loc", [B_LOC, O, HO, WO], f32, kind="ExternalOutput")

    with TileContext(nc) as tc:
        with (
            tc.tile_pool(name="const", bufs=1) as cpool,
            tc.tile_pool(name="xin", bufs=2) as xpool,
            tc.tile_pool(name="gbuf", bufs=12) as gpool,
            tc.tile_pool(name="obuf", bufs=4) as opool,
            tc.tile_pool(name="ps_sel", bufs=3, space="PSUM") as pspool,
            tc.tile_pool(name="ps_out", bufs=2, space="PSUM") as popool,
        ):
            LFULL = HO * 64  # 3968 columns of the padded l' = ho*64+wo axis

            a_r = cpool.tile([72, 360], f32r, tag="a_r")
            nc.sync.dma_start(a_r[:], a_d[:])

            def load_x(x_t, b, h, col0, col1, eng=None):
                """Fill x_t[:, col0:col1] of the unfold view for (b, c-half h)."""
                eng = eng or nc.sync
                for di in range(3):
                    hi = min(col1, H * WIDTH - di * 64 - 2)
                    if hi > col0:
                        ap = _x_window_ap(x_d, b, h, 0, di, hi - col0)
                        ap.offset += col0
                        eng.dma_start(x_t[di * 24 : (di + 1) * 24, col0:hi], ap)
                    if hi < col1:
                        # pad columns feed discarded outputs; fill with
                        # arbitrary valid f32r data to keep reads clean
                        eng.dma_start(
                            x_t[di * 24 : (di + 1) * 24, hi:col1],
                            _x_window_ap(x_d, b, h, 0, 0, col1 - hi),
                        )

            # all unfold loads up front; batch 0 split so tile 0 starts early
            xr_all = []
            for b in range(B_LOC):
                xr_b = []
                for h in range(2):
                    x_t = xpool.tile([72, LFULL], f32r, tag=f"x{h}", name=f"x{h}_{b}")
                    xr_b.append(x_t)
                xr_all.append(xr_b)
            for h in range(2):
                load_x(xr_all[0][h], 0, h, 0, 1024)
            w_r = cpool.tile([GC, NCHUNK, O], f32r, tag="w_r")
            nc.sync.dma_start(w_r[:], w_d[:].rearrange("(k p) o -> p k o", p=GC))
            for h in range(2):
                load_x(xr_all[0][h], 0, h, 1024, LFULL)
            for b in range(1, B_LOC):
                for h in range(2):
                    load_x(xr_all[b][h], b, h, 0, LFULL)

            # greedy ACT/DVE load balancing for PSUM-draining elementwise
            # ops (DVE pays double for squares: bounce + SBUF square)
            eng_busy = {"act": 0.0, "dve": 0.0}

            def square_merged(g_t, ps_s, lt):
                gv = g_t[:, :, :lt]
                pv = ps_s[:, :, :lt]
                if eng_busy["act"] + 1.0 <= eng_busy["dve"] + 2.1:
                    nc.scalar.square(gv, pv)
                    eng_busy["act"] += 1.0
                else:
                    tmp = gpool.tile([GC, 2, 512], f32, tag="sq_tmp")
                    tv = tmp[:, :, :lt]
                    nc.vector.tensor_copy(tv, pv)
                    nc.vector.tensor_mul(gv, tv, tv)
                    eng_busy["dve"] += 2.1

            def out_copy(o_view, ps_view):
                if eng_busy["act"] + 0.9 < eng_busy["dve"] + 0.55:
                    nc.scalar.copy(o_view, ps_view)
                    eng_busy["act"] += 0.9
                else:
                    nc.vector.tensor_copy(o_view, ps_view)
                    eng_busy["dve"] += 0.55

            def do_mains(st):
                """Main matmuls + drain for a tile whose squares are issued."""
                b, ho0, nr, g_ts = st
                lt = nr * 64
                ps_o = popool.tile([O, 512], f32, tag="ps_o", name="ps_o")
                for kk in range(NCHUNK):
                    nc.tensor.matmul(
                        ps_o[:, :lt],
                        w_r[:, kk, :],
                        g_ts[kk // 2][:, kk % 2, :lt],
                        start=(kk == 0),
                        stop=(kk == NCHUNK - 1),
                    )
                # compact to [O, nr*62] so the store uses contiguous chunks
                o_t = opool.tile([O, 8 * WO], f32, tag="o", name="o_t")
                ps_view = ps_o[:, :lt].rearrange("o (r w) -> o r w", w=64)
                o_view = o_t[:, : nr * WO].rearrange("o (r w) -> o r w", w=WO)
                out_copy(o_view, ps_view[:, :, :WO])
                nc.gpsimd.dma_start(
                    o_d[b, :, ho0 : ho0 + nr, :],
                    o_t[:, : nr * WO],
                )

            # one-tile software pipeline skew: issue tile t's selections and
            # squares, then tile t-1's mains — squares get a full tile of
            # slack before the PE needs their output
            pending = None
            for b in range(B_LOC):
                xr = xr_all[b]
                for ho0, nr in ROW_TILES:
                    lt = nr * 64
                    c0 = ho0 * 64
                    g_ts = []
                    for kp in range(NCHUNK // 2):
                        # two 120-row chunks share one 2-bank PSUM tile so one
                        # elementwise op drains both
                        ps_s = pspool.tile([GC, 2, 512], f32, tag="ps_s")
                        for half in range(2):
                            kk = kp * 2 + half
                            h, k = divmod(kk, 3)
                            nc.tensor.matmul(
                                ps_s[:, half, :lt],
                                a_r[:, k * GC : (k + 1) * GC],
                                xr[h][:, c0 : c0 + lt],
                                start=True,
                                stop=True,
                            )
                        g_t = gpool.tile([GC, 2, 512], f32r, tag="g")
                        square_merged(g_t, ps_s, lt)
                        g_ts.append(g_t)
                    if pending is not None:
                        do_mains(pending)
                    pending = (b, ho0, nr, g_ts)
            do_mains(pending)
    nc.compile()
    return nc


@functools.lru_cache(maxsize=1)
def _cached_nc():
    return build_nc()


def kernel(x: np.ndarray, W: np.ndarray, _trace: bool = False):
    x = np.asarray(x, dtype=np.float32)
    W = np.asarray(W, dtype=np.float32)
    AselT, W2T = _build_consts(W)
    x_r = _round_f32r(x)

    nc = _cached_nc()
    in_maps = [
        {
            "x_loc": np.ascontiguousarray(x_r[k * B_LOC : (k + 1) * B_LOC]),
            "aselT": AselT,
            "w2T": W2T,
        }
        for k in range(N_CORES)
    ]
    r = run_bass_kernel_spmd(nc, in_maps, core_ids=list(range(N_CORES)), trace=_trace)
    out = np.concatenate([m["out_loc"] for m in r.results], axis=0)
    if _trace:
        kernel.last_result = r
    return out


if __name__ == "__main__":
    rng = np.random.default_rng(0)
    x = rng.standard_normal((B, C, H, WIDTH), dtype=np.float32)
    W = rng.standard_normal((O, C * 81), dtype=np.float32)
    out = kernel(x, W)
    print("out shape", out.shape, out.dtype)


# revision 40
# speedup vs baseline: 1.7373x; 1.7118x over previous
"""Trainium2 Bass kernel for nn_Fast2Order_DE_Conv.

Math: out[b,o,ho,wo] = sum_{c,i,j} W[o, c*81+i*9+j] * p_i * p_j with
p_i = x[b, c, ho+di, wo+dj] (i = di*3+dj, 3x3 unfold of a 16-channel 64x64
image; output 62x62).

Algorithm: change the quadratic-feature basis from products p_i*p_j to
squares {p_i^2, (p_i+p_j)^2, i<j} (45 per channel, 720 total) and fold the
basis change into W on the host (W2 = W * M^-1).  On-chip, per spatial tile
of 512 locations:

    selection matmul (PE, f32r):  s = AselT.T @ x_unfold   [720 rows]
    square          (ACT/DVE):    g = s^2, PSUM -> SBUF f32r
    main matmul     (PE, f32r):   out += W2T.T @ g, accumulated in PSUM

All matmuls use float32r (fp32 RNE-rounded to 11 mantissa bits, full PE
rate at moving dim >= 256).  Inputs are pre-rounded to the f32r grid on the
host so DMA loads feed the PE directly.  The 3x3 unfold itself is free: it
is expressed in the DMA access pattern (overlapping windows of the padded
l' = ho*64+wo layout).

Sharding: data-parallel over batch, 2 batches per core on 8 cores; W-side
constants are replicated.  Output gathered by simple concatenation.
"""

import functools

import numpy as np

import concourse.bacc as bacc
import concourse.mybir as mybir
from concourse.tile import TileContext
from concourse.bass_utils import run_bass_kernel_spmd

B, C, H, WIDTH = 16, 16, 64, 64
O = 128
HO = WO = 62
N_CORES = 8
B_LOC = B // N_CORES
PAIRS = [(i, j) for i in range(9) for j in range(i, 9)]  # 45
ROW_TILES = [(0, 8), (8, 8), (16, 8), (24, 8), (32, 8), (40, 8), (48, 8), (56, 6)]
NCHUNK = 6  # g chunks of 120 rows (720 total)
GC = 120


def _round_f32r(a: np.ndarray) -> np.ndarray:
    """Round fp32 values to the f32r grid (RNE at 12 low mantissa bits)."""
    a = np.ascontiguousarray(a, dtype=np.float32)
    bits = a.view(np.uint32).astype(np.uint64)
    half, mask = np.uint64(0x800), np.uint64(0xFFF)
    lsb = (bits >> np.uint64(12)) & np.uint64(1)
    out = ((bits + half - np.uint64(1) + lsb) & ~mask).astype(np.uint32)
    return out.view(np.float32).reshape(a.shape)


def _build_consts(Wf: np.ndarray):
    """W (128, 1296) -> (AselT [72, 360] f32, W2T [720, 128] f32, f32r grid)."""
    Wt = np.asarray(Wf, dtype=np.float64).reshape(O, C, 9, 9)
    Wsym = Wt + Wt.transpose(0, 1, 3, 2)
    W2 = np.zeros((O, 720))
    for c in range(C):
        for pi, (i, j) in enumerate(PAIRS):
            f = c * 45 + pi
            if i == j:
                W2[:, f] = Wt[:, c, i, i] - 0.5 * (
                    Wsym[:, c, i, :].sum(-1) - 2.0 * Wt[:, c, i, i]
                )
            else:
                W2[:, f] = 0.5 * Wsym[:, c, i, j]
    # x-row layout on chip: row = i*8 + c_local (i = di*3+dj kernel position)
    AselT = np.zeros((72, 360), dtype=np.float32)
    for cl in range(8):
        for pi, (i, j) in enumerate(PAIRS):
            g = cl * 45 + pi
            AselT[i * 8 + cl, g] += 1.0
            if i != j:
                AselT[j * 8 + cl, g] += 1.0
    W2T = np.ascontiguousarray(W2.T)  # [720, 128]
    return AselT, _round_f32r(W2T)


def _x_window_ap(x_d, b: int, h: int, ho0: int, di: int, lt_load: int):
    """Source AP for one di of the unfold load: (dj, c, l) nesting matching
    target partitions (di*3+dj)*8 + c, free dim = padded l' = ho*64+wo."""
    ap = x_d[b, h * 8 : (h + 1) * 8, ho0 + di, 0:3].unsqueeze(-1)
    v = ap.ap
    v[0] = [1, 3]
    v[1] = [H * WIDTH, 8]
    v[2] = [1, lt_load]
    return ap


def build_nc(reps: int = 1, skew: int = 3):
    """Build the per-core program.  reps>1 wraps the body in an on-chip loop
    (used only for device-time measurement); skew is the software-pipeline
    depth between a tile's selection/squares and its main matmuls."""
    f32, f32r = mybir.dt.float32, mybir.dt.float32r
    nc = bacc.Bacc("TRN2", target_bir_lowering=False)
    x_d = nc.dram_tensor("x_loc", [B_LOC, C, H, WIDTH], f32r, kind="ExternalInput")
    a_d = nc.dram_tensor("aselT", [72, 360], f32r, kind="ExternalInput")
    w_d = nc.dram_tensor("w2T", [720, O], f32r, kind="ExternalInput")
    o_d = nc.dram_tensor("out_loc", [B_LOC, O, HO, WO], f32, kind="ExternalOutput")

    with TileContext(nc) as tc:
        with (
            tc.tile_pool(name="const", bufs=1) as cpool,
            tc.tile_pool(name="xin", bufs=2) as xpool,
            tc.tile_pool(name="gbuf", bufs=3 * (skew + 1) + 2) as gpool,
            tc.tile_pool(name="tmpbuf", bufs=3) as tmppool,
            tc.tile_pool(name="obuf", bufs=4) as opool,
            tc.tile_pool(name="ps_sel", bufs=3, space="PSUM") as pspool,
            tc.tile_pool(name="ps_out", bufs=2, space="PSUM") as popool,
        ):
            LFULL = HO * 64  # 3968 columns of the padded l' = ho*64+wo axis

            a_r = cpool.tile([72, 360], f32r, tag="a_r")
            nc.sync.dma_start(a_r[:], a_d[:])

            def load_x(x_t, b, h, col0, col1, eng=None):
                """Fill x_t[:, col0:col1] of the unfold view for (b, c-half h)."""
                eng = eng or nc.sync
                for di in range(3):
                    hi = min(col1, H * WIDTH - di * 64 - 2)
                    if hi > col0:
                        ap = _x_window_ap(x_d, b, h, 0, di, hi - col0)
                        ap.offset += col0
                        eng.dma_start(x_t[di * 24 : (di + 1) * 24, col0:hi], ap)
                    if hi < col1:
                        # pad columns feed discarded outputs; fill with
                        # arbitrary valid f32r data to keep reads clean
                        eng.dma_start(
                            x_t[di * 24 : (di + 1) * 24, hi:col1],
                            _x_window_ap(x_d, b, h, 0, 0, col1 - hi),
                        )

            # all unfold loads up front; batch 0 split so tile 0 starts early
            xr_all = []
            for b in range(B_LOC):
                xr_b = []
                for h in range(2):
                    x_t = xpool.tile([72, LFULL], f32r, tag=f"x{h}", name=f"x{h}_{b}")
                    xr_b.append(x_t)
                xr_all.append(xr_b)
            for h in range(2):
                load_x(xr_all[0][h], 0, h, 0, 1024)
            w_r = cpool.tile([GC, NCHUNK, O], f32r, tag="w_r")
            nc.sync.dma_start(w_r[:], w_d[:].rearrange("(k p) o -> p k o", p=GC))
            for h in range(2):
                load_x(xr_all[0][h], 0, h, 1024, LFULL)
            for b in range(1, B_LOC):
                for h in range(2):
                    load_x(xr_all[b][h], b, h, 0, LFULL)

            # greedy ACT/DVE load balancing for PSUM-draining elementwise
            # ops (DVE pays double for squares: bounce + SBUF square)
            eng_busy = {"act": 0.0, "dve": 0.0}

            def square_merged(g_t, ps_s, lt):
                gv = g_t[:, :, :lt]
                pv = ps_s[:, :, :lt]
                if eng_busy["act"] + 1.0 <= eng_busy["dve"] + 2.1:
                    nc.scalar.square(gv, pv)
                    eng_busy["act"] += 1.0
                else:
                    tmp = tmppool.tile([GC, 2, 512], f32, tag="sq_tmp")
                    tv = tmp[:, :, :lt]
                    nc.vector.tensor_copy(tv, pv)
                    nc.vector.tensor_mul(gv, tv, tv)
                    eng_busy["dve"] += 2.1

            def out_copy(o_view, ps_view):
                if eng_busy["act"] + 0.9 < eng_busy["dve"] + 0.55:
                    nc.scalar.copy(o_view, ps_view)
                    eng_busy["act"] += 0.9
                else:
                    nc.vector.tensor_copy(o_view, ps_view)
                    eng_busy["dve"] += 0.55

            def do_mains(st):
                """Main matmuls + drain for a tile whose squares are issued."""
                b, ho0, nr, g_ts = st
                lt = nr * 64
                ps_o = popool.tile([O, 512], f32, tag="ps_o", name="ps_o")
                for kk in range(NCHUNK):
                    nc.tensor.matmul(
                        ps_o[:, :lt],
                        w_r[:, kk, :],
                        g_ts[kk // 2][:, kk % 2, :lt],
                        start=(kk == 0),
                        stop=(kk == NCHUNK - 1),
                    )
                # compact to [O, nr*62] so the store uses contiguous chunks
                o_t = opool.tile([O, 8 * WO], f32, tag="o", name="o_t")
                ps_view = ps_o[:, :lt].rearrange("o (r w) -> o r w", w=64)
                o_view = o_t[:, : nr * WO].rearrange("o (r w) -> o r w", w=WO)
                out_copy(o_view, ps_view[:, :, :WO])
                nc.gpsimd.dma_start(
                    o_d[b, :, ho0 : ho0 + nr, :],
                    o_t[:, : nr * WO],
                )

            # HAM warmup: keep the PE busy during the initial DMA wait so the
            # clock gate is at 8/8 when real matmuls start (dummy MMs on the
            # first tile that lands; outputs never read)
            def warmup():
                for i in range(12):
                    ps_w = popool.tile([O, 512], f32, tag="ps_o", name="warm")
                    nc.tensor.matmul(
                        ps_w[:, :360], a_r[:, :128], a_r[:, :360],
                        start=True, stop=True,
                    )

            def body(it=None, unroll=1):
                # software-pipeline skew: issue tile t's selections and
                # squares, then tile (t-skew)'s mains — squares get `skew`
                # tiles of slack before the PE needs their output
                pending = []
                for b in range(B_LOC):
                    xr = xr_all[b]
                    for ho0, nr in ROW_TILES:
                        lt = nr * 64
                        c0 = ho0 * 64
                        g_ts = []
                        for kp in range(NCHUNK // 2):
                            # two 120-row chunks share one 2-bank PSUM tile so
                            # one elementwise op drains both
                            ps_s = pspool.tile(
                                [GC, 2, 512], f32, tag="ps_s", name="ps_s"
                            )
                            for half in range(2):
                                kk = kp * 2 + half
                                h, k = divmod(kk, 3)
                                nc.tensor.matmul(
                                    ps_s[:, half, :lt],
                                    a_r[:, k * GC : (k + 1) * GC],
                                    xr[h][:, c0 : c0 + lt],
                                    start=True,
                                    stop=True,
                                )
                            g_t = gpool.tile(
                                [GC, 2, 512], f32r, tag="g", name="g_t"
                            )
                            square_merged(g_t, ps_s, lt)
                            g_ts.append(g_t)
                        pending.append((b, ho0, nr, g_ts))
                        if len(pending) > skew:
                            do_mains(pending.pop(0))
                for st in pending:
                    do_mains(st)

            warmup()
            if reps == 1:
                body()
            else:
                with tc.For_i(0, reps, 1) as _it:
                    body()
    nc.compile()
    return nc


@functools.lru_cache(maxsize=1)
def _cached_nc():
    return build_nc()


def kernel(x: np.ndarray, W: np.ndarray, _trace: bool = False):
    x = np.asarray(x, dtype=np.float32)
    W = np.asarray(W, dtype=np.float32)
    AselT, W2T = _build_consts(W)
    x_r = _round_f32r(x)

    nc = _cached_nc()
    in_maps = [
        {
            "x_loc": np.ascontiguousarray(x_r[k * B_LOC : (k + 1) * B_LOC]),
            "aselT": AselT,
            "w2T": W2T,
        }
        for k in range(N_CORES)
    ]
    try:
        r = run_bass_kernel_spmd(
            nc, in_maps, core_ids=list(range(N_CORES)), trace=_trace
        )
    except Exception:
        # transient NRT_EXEC_UNIT_UNRECOVERABLE has been observed once on
        # this fabric; a fresh attempt recovers
        r = run_bass_kernel_spmd(
            nc, in_maps, core_ids=list(range(N_CORES)), trace=_trace
        )
    out = np.concatenate([m["out_loc"] for m in r.results], axis=0)
    if _trace:
        kernel.last_result = r
    return out


if __name__ == "__main__":
    rng = np.random.default_rng(0)
    x = rng.standard_normal((B, C, H, WIDTH), dtype=np.float32)
    W = rng.standard_normal((O, C * 81), dtype=np.float32)
    out = kernel(x, W)
    print("out shape", out.shape, out.dtype)
